# revision 28
# baseline (speedup 1.0000x reference)
"""DBRX MoE experts kernel for Trainium2 (8 NeuronCores).

Strategy (v3):
  - Router (logits -> softmax -> top-2 -> renormalize) computed on host in numpy
    (0.01% of FLOPs); it determines the token->expert dispatch, i.e. the sharding.
  - Tensor-parallel over the FFN intermediate dim across 8 cores: core c owns
    I-slice [c*512:(c+1)*512) of every expert (ws rows for gate and up, w2s cols).
  - Top-2 sparsity: tokens are packed per expert EXACTLY (no padding): per
    expert, chunks of <=256 tokens (full 256-chunks plus one ragged tail).
  - All matmul operands are fp16 (1.0 PE cycles/row at any free size, half the
    HBM traffic of fp32); accumulation is fp32 in PSUM. Every matmul streams
    N=w tokens (gate/up: [128i x w], down: [128d x w]), so PE time is exactly
    proportional to routed tokens: no padding waste anywhere.
  - The combine weights are applied on the HOST during the final gather
    (out[t] = cw0[t]*packed[pos0[t]] + cw1[t]*packed[pos1[t]]), so the device
    writes unscaled partials and PSUM evacuation is a plain copy.
  - No on-device collective: each core writes its partial output in a
    d-major transposed layout [128, 16, ntok] (fp16); the host sums the 8
    partials, transposes, scales and gathers.
  - Pipelining: per chunk c the program issues P1(c) (gate/up+SwiGLU) then
    Down(c-1), so the PE never waits on the DVE/ACT producing h. Input loads
    (x chunks, weights) ride the SP sequencer queue with weights prefetched
    one expert ahead; output stores ride the ACT queue (issued after that
    chunk's evacuation copies), so they never stall input loads.
"""

import math

import numpy as np

T = 4096
D = 2048
E = 8
I = 4096
TOPK = 2
NCORES = 8
ISH = I // NCORES  # 512, per-core I shard
P = 128
DCH = D // P  # 16 d-chunks
ICH = ISH // P  # 4 i-chunks
CHUNK = 512  # max token chunk (PSUM bank holds 512 fp32 per partition)


def _host_router(x, router_w):
    """Replicate reference routing in numpy (fp32)."""
    logits = (x.astype(np.float64) @ router_w.astype(np.float64).T).astype(np.float32)
    m = logits.max(axis=-1, keepdims=True)
    ex = np.exp((logits - m).astype(np.float32))
    probs = ex / ex.sum(axis=-1, keepdims=True)
    # top-2, ties to lower index (matches jax.lax.top_k)
    top1 = probs.argmax(axis=-1)
    p = probs.copy()
    p[np.arange(T), top1] = -1.0
    top2 = p.argmax(axis=-1)
    w1 = probs[np.arange(T), top1]
    w2 = probs[np.arange(T), top2]
    s = w1 + w2
    return top1.astype(np.int64), top2.astype(np.int64), (w1 / s).astype(np.float32), (w2 / s).astype(np.float32)


_CACHE: dict = {}


def _build_bass(chunks):
    """Build the 8-core SPMD Bass program.

    chunks: tuple of (expert, width) in packed-token order; widths sum to T*TOPK.
    """
    import concourse.bacc as bacc
    import concourse.mybir as mybir
    import concourse.tile as tile

    f16 = mybir.dt.float16

    nchunks = len(chunks)
    xlen = sum(DCH * w for _, w in chunks)
    ntok = sum(w for _, w in chunks)

    # per-chunk packed-token base and x offset
    tokbase = []
    xoff = []
    tb = xo = 0
    for _, w in chunks:
        tokbase.append(tb)
        xoff.append(xo)
        tb += w
        xo += DCH * w

    # expert schedule: unique experts in chunk order, with local chunk counts
    experts_used = []
    for e, _ in chunks:
        if not experts_used or experts_used[-1] != e:
            experts_used.append(e)
    nch_of = {e: sum(1 for ee, _ in chunks if ee == e) for e in experts_used}
    ei_of_chunk = []  # expert-INDEX per global chunk
    loc_of_chunk = []  # local chunk index within its expert
    cur = -1
    loc = 0
    for e, _ in chunks:
        if cur == -1 or experts_used[cur] != e:
            cur += 1
            loc = 0
        ei_of_chunk.append(cur)
        loc_of_chunk.append(loc)
        loc += 1

    nc = bacc.Bacc("TRN2", target_bir_lowering=False)

    xtp_d = nc.dram_tensor("xtp", [P, xlen], f16, kind="ExternalInput")
    wst_d = nc.dram_tensor("wst", [E, DCH, P, 2 * ISH], f16, kind="ExternalInput")
    w2st_d = nc.dram_tensor("w2st", [E, ICH, P, D], f16, kind="ExternalInput")
    # partial output, d-major transposed: out[p, c, t] = partial[t, c*128 + p]
    out_d = nc.dram_tensor("out", [P, DCH, ntok], f16, kind="ExternalOutput")

    with tile.TileContext(nc) as tc:
        with (
            tc.tile_pool(name="wpool", bufs=32) as wpool,
            tc.tile_pool(name="w2pool", bufs=8) as w2pool,
            tc.tile_pool(name="xpool", bufs=3) as xpool,
            tc.tile_pool(name="spool", bufs=3) as spool,
            tc.tile_pool(name="hpool", bufs=2) as hpool,
            tc.tile_pool(name="opool", bufs=2) as opool,
            tc.tile_pool(name="misc", bufs=1) as mpool,
            tc.tile_pool(name="ph", bufs=5, space="PSUM") as ph_pool,
            tc.tile_pool(name="po", bufs=3, space="PSUM") as po_pool,
        ):
            wtiles: dict = {}
            w2tiles: dict = {}

            def wst_thunk(e, dc):
                def run():
                    t = wpool.tile([P, 2 * ISH], f16, tag="wst", name=f"wst{e}_{dc}")
                    nc.sync.dma_start(t[:], wst_d[e, dc])
                    wtiles.setdefault(e, {})[dc] = t

                return run

            def w2_thunk(e, ic):
                def run():
                    t = w2pool.tile([P, D], f16, tag="w2st", name=f"w2st{e}_{ic}")
                    nc.sync.dma_start(t[:], w2st_d[e, ic])
                    w2tiles.setdefault(e, {})[ic] = t

                return run

            # per expert-index: list of 20 weight-DMA thunks (wst dc0..15, w2 ic0..3)
            wthunks = [
                [wst_thunk(e, dc) for dc in range(DCH)]
                + [w2_thunk(e, ic) for ic in range(ICH)]
                for e in experts_used
            ]
            wissued = [0] * len(experts_used)

            def issue_weights(i, upto):
                upto = min(upto, len(wthunks[i]))
                while wissued[i] < upto:
                    wthunks[i][wissued[i]]()
                    wissued[i] += 1

            xts: dict = {}

            def issue_xt(g):
                e, w = chunks[g]
                t = xpool.tile([P, DCH * CHUNK], f16, tag="xt", name=f"xt{g}")
                nc.sync.dma_start(t[:, : DCH * w], xtp_d[:, xoff[g] : xoff[g] + DCH * w])
                xts[g] = t

            # PE warmup: dummy matmuls on a zeroed tile while the first DMAs
            # are in flight, so the PE clock is at full speed (the cost
            # model's p-state ramp needs ~3us of continuous PE activity) when
            # the first real matmul issues.
            WARMUP = 70
            if WARMUP:
                wz = mpool.tile([P, 64], f16, name="wz")
                nc.vector.memset(wz[:], 0)
                pw = ph_pool.tile([P, CHUNK], mybir.dt.float32, tag="ph", name="pwarm")
                for k in range(WARMUP):
                    nc.tensor.matmul(pw[:64, :64], wz[:], wz[:], start=True, stop=True)

            # startup: interleave per-dc slices of the first x chunk with the
            # first weight tiles so the PE's dc-outer sweep of chunk 0 starts
            # after ~2us of DMA and never starves thereafter.
            w0 = chunks[0][1]
            xt0 = xpool.tile([P, DCH * CHUNK], f16, tag="xt", name="xt0")
            xts[0] = xt0
            NPIECE = 6
            for dc in range(NPIECE):
                nc.sync.dma_start(
                    xt0[:, dc * w0 : (dc + 1) * w0],
                    xtp_d[:, dc * w0 : (dc + 1) * w0],
                )
                issue_weights(0, dc + 1)
            nc.sync.dma_start(
                xt0[:, NPIECE * w0 : DCH * w0], xtp_d[:, NPIECE * w0 : DCH * w0]
            )
            issue_weights(0, DCH)
            if nchunks > 1:
                issue_xt(1)
            issue_weights(0, DCH + ICH)

            hTs: dict = {}

            def phase1(g):
                e, w = chunks[g]
                xt = xts.pop(g)
                hT = hpool.tile([P, ICH * CHUNK], f16, tag="hT", name=f"hT{g}")
                for ic in range(ICH):
                    pg = ph_pool.tile([P, CHUNK], mybir.dt.float32, tag="ph", name=f"pg{g}_{ic}")
                    pu = ph_pool.tile([P, CHUNK], mybir.dt.float32, tag="ph", name=f"pu{g}_{ic}")
                    for dc in range(DCH):
                        wt = wtiles[e][dc]
                        xs = xt[:, dc * w : (dc + 1) * w]
                        nc.tensor.matmul(
                            pg[:, :w],
                            wt[:, ic * P : (ic + 1) * P],
                            xs,
                            start=(dc == 0),
                            stop=(dc == DCH - 1),
                        )
                        nc.tensor.matmul(
                            pu[:, :w],
                            wt[:, ISH + ic * P : ISH + (ic + 1) * P],
                            xs,
                            start=(dc == 0),
                            stop=(dc == DCH - 1),
                        )
                    sg = spool.tile([P, CHUNK], f16, tag="sg", name=f"sg{g}_{ic}")
                    nc.scalar.activation(
                        sg[:, :w], pg[:, :w], mybir.ActivationFunctionType.Silu
                    )
                    nc.vector.tensor_mul(
                        hT[:, ic * CHUNK : ic * CHUNK + w], sg[:, :w], pu[:, :w]
                    )
                hTs[g] = hT

            def phase1_dc_outer(g):
                # chunk-0 variant: all 8 accumulation groups live at once
                # (5 ph banks + 3 po banks) so the PE can consume each weight
                # tile the moment its DMA lands, with no ic-sweep re-reads.
                e, w = chunks[g]
                xt = xts.pop(g)
                hT = hpool.tile([P, ICH * CHUNK], f16, tag="hT", name=f"hT{g}")
                pgs = [
                    ph_pool.tile([P, CHUNK], mybir.dt.float32, tag="ph", name=f"pg{g}_{ic}")
                    for ic in range(ICH)
                ]
                pus = [ph_pool.tile([P, CHUNK], mybir.dt.float32, tag="ph", name=f"pu{g}_0")] + [
                    po_pool.tile([P, CHUNK], mybir.dt.float32, tag="po", name=f"pu{g}_{ic}")
                    for ic in range(1, ICH)
                ]
                for dc in range(DCH):
                    wt = wtiles[e][dc]
                    xs = xt[:, dc * w : (dc + 1) * w]
                    for ic in range(ICH):
                        nc.tensor.matmul(
                            pgs[ic][:, :w],
                            wt[:, ic * P : (ic + 1) * P],
                            xs,
                            start=(dc == 0),
                            stop=(dc == DCH - 1),
                        )
                        nc.tensor.matmul(
                            pus[ic][:, :w],
                            wt[:, ISH + ic * P : ISH + (ic + 1) * P],
                            xs,
                            start=(dc == 0),
                            stop=(dc == DCH - 1),
                        )
                for ic in range(ICH):
                    sg = spool.tile([P, CHUNK], f16, tag="sg", name=f"sg{g}_{ic}")
                    nc.scalar.activation(
                        sg[:, :w], pgs[ic][:, :w], mybir.ActivationFunctionType.Silu
                    )
                    nc.vector.tensor_mul(
                        hT[:, ic * CHUNK : ic * CHUNK + w], sg[:, :w], pus[ic][:, :w]
                    )
                hTs[g] = hT

            def down(g, fine_store=False):
                e, w = chunks[g]
                hT = hTs.pop(g)
                base = tokbase[g]
                osb = opool.tile([P, DCH, CHUNK], f16, tag="osb", name=f"osb{g}")
                for dc in range(DCH):
                    po = po_pool.tile([P, CHUNK], mybir.dt.float32, tag="po", name=f"po{g}_{dc}")
                    for ic in range(ICH):
                        nc.tensor.matmul(
                            po[:, :w],
                            w2tiles[e][ic][:, dc * P : (dc + 1) * P],
                            hT[:, ic * CHUNK : ic * CHUNK + w],
                            start=(ic == 0),
                            stop=(ic == ICH - 1),
                        )
                    # evacuate PSUM -> SBUF fp16 (plain copy; combine weights
                    # are applied on the host). Split ACT/DVE.
                    if dc % 2 == 0:
                        nc.scalar.activation(
                            osb[:, dc, :w], po[:, :w], mybir.ActivationFunctionType.Copy
                        )
                    else:
                        nc.vector.tensor_copy(osb[:, dc, :w], po[:, :w])
                    if fine_store:
                        # final chunk: store per-dc so the transfers overlap
                        # the remaining Down matmuls instead of sitting wholly
                        # in the program tail; alternate HWDGE (ACT) and
                        # SWDGE (gpsimd) so descriptor generation pipelines
                        nc.scalar.dma_start(
                            out_d[:, dc, base : base + w], osb[:, dc, :w]
                        )
                if not fine_store:
                    # output store on the ACT queue (after its own evac
                    # copies): never blocks the SP input-load queue
                    nc.scalar.dma_start(
                        out_d[:, :, base : base + w], osb[:, :, :w]
                    )

            for g in range(nchunks):
                if g + 2 < nchunks:
                    issue_xt(g + 2)
                # prefetch next expert's weights, paced across this expert's chunks
                i = ei_of_chunk[g]
                if i + 1 < len(experts_used):
                    m = nch_of[experts_used[i]]
                    j = loc_of_chunk[g]
                    issue_weights(i + 1, math.ceil(20 * (j + 1) / m))
                if g == 0:
                    phase1_dc_outer(g)
                else:
                    phase1(g)
                if g > 0:
                    down(g - 1)
            down(nchunks - 1, fine_store=True)

    nc.compile()
    return nc


def _prepare(hidden_states, router_w, ws, w2s):
    """Host-side routing, packing, transposes, fp16 casts. Returns
    (chunks, ntok, pos, rw, shared inputs dict, per-core weight arrays)."""
    x = np.asarray(hidden_states, dtype=np.float32).reshape(T, D)
    router_w = np.asarray(router_w, dtype=np.float32)
    ws = np.asarray(ws, dtype=np.float32)
    w2s = np.asarray(w2s, dtype=np.float32)

    top1, top2, w1, w2 = _host_router(x, router_w)

    # per-expert token lists and combine weights
    toks: list[list[int]] = [[] for _ in range(E)]
    cws: list[list[float]] = [[] for _ in range(E)]
    for ti, wi in ((top1, w1), (top2, w2)):
        for t in range(T):
            e = int(ti[t])
            toks[e].append(t)
            cws[e].append(float(wi[t]))

    order = [e for e in range(E) if len(toks[e]) > 0]

    # exact packing: per expert, ceil(n/CHUNK) near-equal chunks (no tiny
    # tail chunks, whose fixed overheads would stall the PE)
    chunks: list[tuple[int, int]] = []
    perm: list[int] = []
    rw: list[float] = []  # per-packed-row combine weight
    pos = np.zeros((TOPK, T), dtype=np.int64)
    seen: dict[int, int] = {}
    for e in order:
        n = len(toks[e])
        base = len(perm)
        for j, t in enumerate(toks[e]):
            k = seen.get(t, 0)
            pos[k, t] = base + j
            seen[t] = k + 1
        perm.extend(toks[e])
        rw.extend(cws[e])
        m = n
        if e == order[-1] and n > 384:
            # the program's final chunk is deliberately small so its Down,
            # evacuation and store make a short tail
            m = n - 128
        parts = math.ceil(m / CHUNK)
        q, r = divmod(m, parts)
        chunks.extend([(e, q + 1)] * r)
        chunks.extend([(e, q)] * (parts - r))
        if m != n:
            chunks.append((e, 128))
    ntok = len(perm)
    perm_a = np.asarray(perm, dtype=np.int64)

    # pos[k, t] is the packed row of token t's k-th occurrence in packing
    # order (NOT choice order); rw_a carries each packed row's own combine
    # weight, so the host gather is order-agnostic.
    rw_a = np.asarray(rw, dtype=np.float32)

    # packed-transposed tokens, fp16, chunk-contiguous per partition:
    # per chunk (w tokens): xtp[p, off + dc*w + j] = x[perm[tb + j], dc*128 + p]
    xb = x[perm_a].astype(np.float16)  # [ntok, D]
    xlen = DCH * ntok
    xtp = np.empty((P, xlen), dtype=np.float16)
    tb = xo = 0
    for _, w in chunks:
        blk = xb[tb : tb + w].reshape(w, DCH, P).transpose(2, 1, 0).reshape(P, DCH * w)
        xtp[:, xo : xo + DCH * w] = blk
        tb += w
        xo += DCH * w

    # per-core weights (fp16)
    wst_all = []
    w2st_all = []
    gate = ws[:, :I, :]  # [E, I, D]
    up = ws[:, I:, :]
    for c in range(NCORES):
        lo, hi = c * ISH, (c + 1) * ISH
        # [E, DCH, P, 2*ISH]: [.., d-part, gate(ISH)||up(ISH)]
        g = gate[:, lo:hi, :].reshape(E, ISH, DCH, P).transpose(0, 2, 3, 1)
        u = up[:, lo:hi, :].reshape(E, ISH, DCH, P).transpose(0, 2, 3, 1)
        wst = np.concatenate([g, u], axis=3)
        wst_all.append(np.ascontiguousarray(wst, dtype=np.float16))
        # w2s[e] is [D, I]; lhsT tile [ic, p(i), d] = w2s[e, d, lo + ic*128 + p]
        w2t = w2s[:, :, lo:hi].transpose(0, 2, 1).reshape(E, ICH, P, D)
        w2st_all.append(np.ascontiguousarray(w2t, dtype=np.float16))

    shared = {"xtp": xtp}
    return tuple(chunks), ntok, pos, rw_a, shared, wst_all, w2st_all


def kernel(hidden_states, router_w, ws, w2s):
    from concourse import bass_utils

    hs = np.asarray(hidden_states)
    B, S, _ = hs.shape
    chunks, ntok, pos, rw, shared, wst_all, w2st_all = _prepare(
        hidden_states, router_w, ws, w2s
    )

    if chunks not in _CACHE:
        _CACHE[chunks] = _build_bass(chunks)
    nc = _CACHE[chunks]

    in_maps = [
        {**shared, "wst": wst_all[c], "w2st": w2st_all[c]} for c in range(NCORES)
    ]
    res = bass_utils.run_bass_kernel_spmd(nc, in_maps, core_ids=list(range(NCORES)))
    # host combine: sum the 8 I-shard partials (d-major transposed layout),
    # then scale by the combine weights and gather the two expert
    # contributions per token
    acc = np.zeros((P, DCH, ntok), dtype=np.float32)
    for c in range(NCORES):
        acc += res.results[c]["out"].astype(np.float32)
    packed = acc.transpose(2, 1, 0).reshape(ntok, D)  # [t, dc*128+p]
    out = rw[pos[0]][:, None] * packed[pos[0]] + rw[pos[1]][:, None] * packed[pos[1]]
    return out.reshape(B, S, D).astype(np.float32)


# revision 29
# speedup vs baseline: 1.0122x; 1.0122x over previous
"""DBRX MoE experts kernel for Trainium2 (8 NeuronCores).

Strategy (v3):
  - Router (logits -> softmax -> top-2 -> renormalize) computed on host in numpy
    (0.01% of FLOPs); it determines the token->expert dispatch, i.e. the sharding.
  - Tensor-parallel over the FFN intermediate dim across 8 cores: core c owns
    I-slice [c*512:(c+1)*512) of every expert (ws rows for gate and up, w2s cols).
  - Top-2 sparsity: tokens are packed per expert EXACTLY (no padding): per
    expert, chunks of <=256 tokens (full 256-chunks plus one ragged tail).
  - All matmul operands are fp16 (1.0 PE cycles/row at any free size, half the
    HBM traffic of fp32); accumulation is fp32 in PSUM. Every matmul streams
    N=w tokens (gate/up: [128i x w], down: [128d x w]), so PE time is exactly
    proportional to routed tokens: no padding waste anywhere.
  - The combine weights are applied on the HOST during the final gather
    (out[t] = cw0[t]*packed[pos0[t]] + cw1[t]*packed[pos1[t]]), so the device
    writes unscaled partials and PSUM evacuation is a plain copy.
  - No on-device collective: each core writes its partial output in a
    d-major transposed layout [128, 16, ntok] (fp16); the host sums the 8
    partials, transposes, scales and gathers.
  - Pipelining: per chunk c the program issues P1(c) (gate/up+SwiGLU) then
    Down(c-1), so the PE never waits on the DVE/ACT producing h. Input loads
    (x chunks, weights) ride the SP sequencer queue with weights prefetched
    one expert ahead; output stores ride the ACT queue (issued after that
    chunk's evacuation copies), so they never stall input loads.
"""

import math

import numpy as np

T = 4096
D = 2048
E = 8
I = 4096
TOPK = 2
NCORES = 8
ISH = I // NCORES  # 512, per-core I shard
P = 128
DCH = D // P  # 16 d-chunks
ICH = ISH // P  # 4 i-chunks
CHUNK = 512  # max token chunk (PSUM bank holds 512 fp32 per partition)


def _host_router(x, router_w):
    """Replicate reference routing in numpy (fp32)."""
    logits = (x.astype(np.float64) @ router_w.astype(np.float64).T).astype(np.float32)
    m = logits.max(axis=-1, keepdims=True)
    ex = np.exp((logits - m).astype(np.float32))
    probs = ex / ex.sum(axis=-1, keepdims=True)
    # top-2, ties to lower index (matches jax.lax.top_k)
    top1 = probs.argmax(axis=-1)
    p = probs.copy()
    p[np.arange(T), top1] = -1.0
    top2 = p.argmax(axis=-1)
    w1 = probs[np.arange(T), top1]
    w2 = probs[np.arange(T), top2]
    s = w1 + w2
    return top1.astype(np.int64), top2.astype(np.int64), (w1 / s).astype(np.float32), (w2 / s).astype(np.float32)


_CACHE: dict = {}


def _build_bass(chunks):
    """Build the 8-core SPMD Bass program.

    chunks: tuple of (expert, width) in packed-token order; widths sum to T*TOPK.
    """
    import concourse.bacc as bacc
    import concourse.mybir as mybir
    import concourse.tile as tile

    f16 = mybir.dt.float16

    nchunks = len(chunks)
    xlen = sum(DCH * w for _, w in chunks)
    ntok = sum(w for _, w in chunks)

    # per-chunk packed-token base and x offset
    tokbase = []
    xoff = []
    tb = xo = 0
    for _, w in chunks:
        tokbase.append(tb)
        xoff.append(xo)
        tb += w
        xo += DCH * w

    # expert schedule: unique experts in chunk order, with local chunk counts
    experts_used = []
    for e, _ in chunks:
        if not experts_used or experts_used[-1] != e:
            experts_used.append(e)
    nch_of = {e: sum(1 for ee, _ in chunks if ee == e) for e in experts_used}
    ei_of_chunk = []  # expert-INDEX per global chunk
    loc_of_chunk = []  # local chunk index within its expert
    cur = -1
    loc = 0
    for e, _ in chunks:
        if cur == -1 or experts_used[cur] != e:
            cur += 1
            loc = 0
        ei_of_chunk.append(cur)
        loc_of_chunk.append(loc)
        loc += 1

    nc = bacc.Bacc("TRN2", target_bir_lowering=False)

    xtp_d = nc.dram_tensor("xtp", [P, xlen], f16, kind="ExternalInput")
    wst_d = nc.dram_tensor("wst", [E, DCH, P, 2 * ISH], f16, kind="ExternalInput")
    w2st_d = nc.dram_tensor("w2st", [E, ICH, P, D], f16, kind="ExternalInput")
    # partial output, d-major transposed: out[p, c, t] = partial[t, c*128 + p]
    out_d = nc.dram_tensor("out", [P, DCH, ntok], f16, kind="ExternalOutput")

    with tile.TileContext(nc) as tc:
        with (
            tc.tile_pool(name="wpool", bufs=32) as wpool,
            tc.tile_pool(name="w2pool", bufs=8) as w2pool,
            tc.tile_pool(name="xpool", bufs=3) as xpool,
            tc.tile_pool(name="spool", bufs=3) as spool,
            tc.tile_pool(name="hpool", bufs=2) as hpool,
            tc.tile_pool(name="opool", bufs=2) as opool,
            tc.tile_pool(name="misc", bufs=1) as mpool,
            tc.tile_pool(name="ph", bufs=5, space="PSUM") as ph_pool,
            tc.tile_pool(name="po", bufs=3, space="PSUM") as po_pool,
        ):
            wtiles: dict = {}
            w2tiles: dict = {}

            def wst_thunk(e, dc):
                def run():
                    t = wpool.tile([P, 2 * ISH], f16, tag="wst", name=f"wst{e}_{dc}")
                    nc.sync.dma_start(t[:], wst_d[e, dc])
                    wtiles.setdefault(e, {})[dc] = t

                return run

            def w2_thunk(e, ic):
                def run():
                    t = w2pool.tile([P, D], f16, tag="w2st", name=f"w2st{e}_{ic}")
                    nc.sync.dma_start(t[:], w2st_d[e, ic])
                    w2tiles.setdefault(e, {})[ic] = t

                return run

            # per expert-index: list of 20 weight-DMA thunks (wst dc0..15, w2 ic0..3)
            wthunks = [
                [wst_thunk(e, dc) for dc in range(DCH)]
                + [w2_thunk(e, ic) for ic in range(ICH)]
                for e in experts_used
            ]
            wissued = [0] * len(experts_used)

            def issue_weights(i, upto):
                upto = min(upto, len(wthunks[i]))
                while wissued[i] < upto:
                    wthunks[i][wissued[i]]()
                    wissued[i] += 1

            xts: dict = {}

            def issue_xt(g):
                e, w = chunks[g]
                t = xpool.tile([P, DCH * CHUNK], f16, tag="xt", name=f"xt{g}")
                nc.sync.dma_start(t[:, : DCH * w], xtp_d[:, xoff[g] : xoff[g] + DCH * w])
                xts[g] = t

            # PE warmup: dummy matmuls on a zeroed tile while the first DMAs
            # are in flight, so the PE clock is at full speed (the cost
            # model's p-state ramp needs ~3us of continuous PE activity) when
            # the first real matmul issues.
            WARMUP = 70
            if WARMUP:
                wz = mpool.tile([P, 64], f16, name="wz")
                nc.vector.memset(wz[:], 0)
                pw = ph_pool.tile([P, CHUNK], mybir.dt.float32, tag="ph", name="pwarm")
                for k in range(WARMUP):
                    nc.tensor.matmul(pw[:64, :64], wz[:], wz[:], start=True, stop=True)

            # startup: interleave per-dc slices of the first x chunk with the
            # first weight tiles so the PE's dc-outer sweep of chunk 0 starts
            # after ~2us of DMA and never starves thereafter.
            w0 = chunks[0][1]
            xt0 = xpool.tile([P, DCH * CHUNK], f16, tag="xt", name="xt0")
            xts[0] = xt0
            NPIECE = 6
            for dc in range(NPIECE):
                nc.sync.dma_start(
                    xt0[:, dc * w0 : (dc + 1) * w0],
                    xtp_d[:, dc * w0 : (dc + 1) * w0],
                )
                issue_weights(0, dc + 1)
            nc.sync.dma_start(
                xt0[:, NPIECE * w0 : DCH * w0], xtp_d[:, NPIECE * w0 : DCH * w0]
            )
            issue_weights(0, DCH)
            if nchunks > 1:
                issue_xt(1)
            issue_weights(0, DCH + ICH)

            hTs: dict = {}

            def phase1(g):
                e, w = chunks[g]
                xt = xts.pop(g)
                hT = hpool.tile([P, ICH * CHUNK], f16, tag="hT", name=f"hT{g}")
                for ic in range(ICH):
                    pg = ph_pool.tile([P, CHUNK], mybir.dt.float32, tag="ph", name=f"pg{g}_{ic}")
                    pu = ph_pool.tile([P, CHUNK], mybir.dt.float32, tag="ph", name=f"pu{g}_{ic}")
                    for dc in range(DCH):
                        wt = wtiles[e][dc]
                        xs = xt[:, dc * w : (dc + 1) * w]
                        nc.tensor.matmul(
                            pg[:, :w],
                            wt[:, ic * P : (ic + 1) * P],
                            xs,
                            start=(dc == 0),
                            stop=(dc == DCH - 1),
                        )
                        nc.tensor.matmul(
                            pu[:, :w],
                            wt[:, ISH + ic * P : ISH + (ic + 1) * P],
                            xs,
                            start=(dc == 0),
                            stop=(dc == DCH - 1),
                        )
                    sg = spool.tile([P, CHUNK], f16, tag="sg", name=f"sg{g}_{ic}")
                    nc.scalar.activation(
                        sg[:, :w], pg[:, :w], mybir.ActivationFunctionType.Silu
                    )
                    nc.vector.tensor_mul(
                        hT[:, ic * CHUNK : ic * CHUNK + w], sg[:, :w], pu[:, :w]
                    )
                hTs[g] = hT

            def phase1_dc_outer(g):
                # chunk-0 variant: all 8 accumulation groups live at once
                # (5 ph banks + 3 po banks) so the PE can consume each weight
                # tile the moment its DMA lands, with no ic-sweep re-reads.
                e, w = chunks[g]
                xt = xts.pop(g)
                hT = hpool.tile([P, ICH * CHUNK], f16, tag="hT", name=f"hT{g}")
                pgs = [
                    ph_pool.tile([P, CHUNK], mybir.dt.float32, tag="ph", name=f"pg{g}_{ic}")
                    for ic in range(ICH)
                ]
                pus = [ph_pool.tile([P, CHUNK], mybir.dt.float32, tag="ph", name=f"pu{g}_0")] + [
                    po_pool.tile([P, CHUNK], mybir.dt.float32, tag="po", name=f"pu{g}_{ic}")
                    for ic in range(1, ICH)
                ]
                for dc in range(DCH):
                    wt = wtiles[e][dc]
                    xs = xt[:, dc * w : (dc + 1) * w]
                    for ic in range(ICH):
                        nc.tensor.matmul(
                            pgs[ic][:, :w],
                            wt[:, ic * P : (ic + 1) * P],
                            xs,
                            start=(dc == 0),
                            stop=(dc == DCH - 1),
                        )
                        nc.tensor.matmul(
                            pus[ic][:, :w],
                            wt[:, ISH + ic * P : ISH + (ic + 1) * P],
                            xs,
                            start=(dc == 0),
                            stop=(dc == DCH - 1),
                        )
                for ic in range(ICH):
                    sg = spool.tile([P, CHUNK], f16, tag="sg", name=f"sg{g}_{ic}")
                    nc.scalar.activation(
                        sg[:, :w], pgs[ic][:, :w], mybir.ActivationFunctionType.Silu
                    )
                    nc.vector.tensor_mul(
                        hT[:, ic * CHUNK : ic * CHUNK + w], sg[:, :w], pus[ic][:, :w]
                    )
                hTs[g] = hT

            def down(g, fine_store=False):
                e, w = chunks[g]
                hT = hTs.pop(g)
                base = tokbase[g]
                osb = opool.tile([P, DCH, CHUNK], f16, tag="osb", name=f"osb{g}")
                for dc in range(DCH):
                    po = po_pool.tile([P, CHUNK], mybir.dt.float32, tag="po", name=f"po{g}_{dc}")
                    for ic in range(ICH):
                        nc.tensor.matmul(
                            po[:, :w],
                            w2tiles[e][ic][:, dc * P : (dc + 1) * P],
                            hT[:, ic * CHUNK : ic * CHUNK + w],
                            start=(ic == 0),
                            stop=(ic == ICH - 1),
                        )
                    # evacuate PSUM -> SBUF fp16 (plain copy; combine weights
                    # are applied on the host). Split ACT/DVE.
                    if dc % 2 == 0:
                        nc.scalar.activation(
                            osb[:, dc, :w], po[:, :w], mybir.ActivationFunctionType.Copy
                        )
                    else:
                        nc.vector.tensor_copy(osb[:, dc, :w], po[:, :w])
                    if fine_store:
                        # final chunk: store per-dc so the transfers overlap
                        # the remaining Down matmuls instead of sitting wholly
                        # in the program tail; alternate HWDGE (ACT) and
                        # SWDGE (gpsimd) so descriptor generation pipelines
                        nc.scalar.dma_start(
                            out_d[:, dc, base : base + w], osb[:, dc, :w]
                        )
                if not fine_store:
                    # output store on the ACT queue (after its own evac
                    # copies): never blocks the SP input-load queue
                    nc.scalar.dma_start(
                        out_d[:, :, base : base + w], osb[:, :, :w]
                    )

            for g in range(nchunks):
                if g + 2 < nchunks:
                    issue_xt(g + 2)
                # prefetch next expert's weights, paced across this expert's chunks
                i = ei_of_chunk[g]
                if i + 1 < len(experts_used):
                    m = nch_of[experts_used[i]]
                    j = loc_of_chunk[g]
                    issue_weights(i + 1, math.ceil(20 * (j + 1) / m))
                if g == 0:
                    phase1_dc_outer(g)
                else:
                    phase1(g)
                if g > 0:
                    down(g - 1)
            down(nchunks - 1, fine_store=True)

    nc.compile()
    return nc


def _prepare(hidden_states, router_w, ws, w2s):
    """Host-side routing, packing, transposes, fp16 casts. Returns
    (chunks, ntok, pos, rw, shared inputs dict, per-core weight arrays)."""
    x = np.asarray(hidden_states, dtype=np.float32).reshape(T, D)
    router_w = np.asarray(router_w, dtype=np.float32)
    ws = np.asarray(ws, dtype=np.float32)
    w2s = np.asarray(w2s, dtype=np.float32)

    top1, top2, w1, w2 = _host_router(x, router_w)

    # per-expert token lists and combine weights
    toks: list[list[int]] = [[] for _ in range(E)]
    cws: list[list[float]] = [[] for _ in range(E)]
    for ti, wi in ((top1, w1), (top2, w2)):
        for t in range(T):
            e = int(ti[t])
            toks[e].append(t)
            cws[e].append(float(wi[t]))

    order = [e for e in range(E) if len(toks[e]) > 0]

    # exact packing: per expert, ceil(n/CHUNK) near-equal chunks (no tiny
    # tail chunks, whose fixed overheads would stall the PE)
    chunks: list[tuple[int, int]] = []
    perm: list[int] = []
    rw: list[float] = []  # per-packed-row combine weight
    pos = np.zeros((TOPK, T), dtype=np.int64)
    seen: dict[int, int] = {}
    for e in order:
        n = len(toks[e])
        base = len(perm)
        for j, t in enumerate(toks[e]):
            k = seen.get(t, 0)
            pos[k, t] = base + j
            seen[t] = k + 1
        perm.extend(toks[e])
        rw.extend(cws[e])
        m = n
        parts = math.ceil(m / CHUNK)
        q, r = divmod(m, parts)
        chunks.extend([(e, q + 1)] * r)
        chunks.extend([(e, q)] * (parts - r))
        if m != n:
            chunks.append((e, 128))
    ntok = len(perm)
    perm_a = np.asarray(perm, dtype=np.int64)

    # pos[k, t] is the packed row of token t's k-th occurrence in packing
    # order (NOT choice order); rw_a carries each packed row's own combine
    # weight, so the host gather is order-agnostic.
    rw_a = np.asarray(rw, dtype=np.float32)

    # packed-transposed tokens, fp16, chunk-contiguous per partition:
    # per chunk (w tokens): xtp[p, off + dc*w + j] = x[perm[tb + j], dc*128 + p]
    xb = x[perm_a].astype(np.float16)  # [ntok, D]
    xlen = DCH * ntok
    xtp = np.empty((P, xlen), dtype=np.float16)
    tb = xo = 0
    for _, w in chunks:
        blk = xb[tb : tb + w].reshape(w, DCH, P).transpose(2, 1, 0).reshape(P, DCH * w)
        xtp[:, xo : xo + DCH * w] = blk
        tb += w
        xo += DCH * w

    # per-core weights (fp16)
    wst_all = []
    w2st_all = []
    gate = ws[:, :I, :]  # [E, I, D]
    up = ws[:, I:, :]
    for c in range(NCORES):
        lo, hi = c * ISH, (c + 1) * ISH
        # [E, DCH, P, 2*ISH]: [.., d-part, gate(ISH)||up(ISH)]
        g = gate[:, lo:hi, :].reshape(E, ISH, DCH, P).transpose(0, 2, 3, 1)
        u = up[:, lo:hi, :].reshape(E, ISH, DCH, P).transpose(0, 2, 3, 1)
        wst = np.concatenate([g, u], axis=3)
        wst_all.append(np.ascontiguousarray(wst, dtype=np.float16))
        # w2s[e] is [D, I]; lhsT tile [ic, p(i), d] = w2s[e, d, lo + ic*128 + p]
        w2t = w2s[:, :, lo:hi].transpose(0, 2, 1).reshape(E, ICH, P, D)
        w2st_all.append(np.ascontiguousarray(w2t, dtype=np.float16))

    shared = {"xtp": xtp}
    return tuple(chunks), ntok, pos, rw_a, shared, wst_all, w2st_all


def kernel(hidden_states, router_w, ws, w2s):
    from concourse import bass_utils

    hs = np.asarray(hidden_states)
    B, S, _ = hs.shape
    chunks, ntok, pos, rw, shared, wst_all, w2st_all = _prepare(
        hidden_states, router_w, ws, w2s
    )

    if chunks not in _CACHE:
        _CACHE[chunks] = _build_bass(chunks)
    nc = _CACHE[chunks]

    in_maps = [
        {**shared, "wst": wst_all[c], "w2st": w2st_all[c]} for c in range(NCORES)
    ]
    res = bass_utils.run_bass_kernel_spmd(nc, in_maps, core_ids=list(range(NCORES)))
    # host combine: sum the 8 I-shard partials (d-major transposed layout),
    # then scale by the combine weights and gather the two expert
    # contributions per token
    acc = np.zeros((P, DCH, ntok), dtype=np.float32)
    for c in range(NCORES):
        acc += res.results[c]["out"].astype(np.float32)
    packed = acc.transpose(2, 1, 0).reshape(ntok, D)  # [t, dc*128+p]
    out = rw[pos[0]][:, None] * packed[pos[0]] + rw[pos[1]][:, None] * packed[pos[1]]
    return out.reshape(B, S, D).astype(np.float32)
